# revision 55
# baseline (speedup 1.0000x reference)
"""Trainium2 Bass kernel for nn_MiddleOut (gnn_message_passing).

Math (reference):
    out[b,r] = mean_p[ m[b,p] * (my@Wm.T + bias + peer[b,p]@Wp.T + m[b,p]*wm)[r] ]
Collapses to (P = #peers):
    s1[b] = sum_p m[b,p];  s2[b] = sum_p m[b,p]^2
    z[b,l] = sum_p m[b,p] * peer[b,p,l]
    out = s1*(my@WmT') + z@WpT' + s2*wm' + s1*bias'     (W' = W/P)

Sharding: pure data parallel over batch across 8 cores.

On-device strategy per core (Bc=2048 rows, 16 tiles of 128):
  - One fused ~1.1MB DMA per tile: [x fp8e3 | myT bf16 | m f32] packed per
    partition lane on host; x host-permuted to [(b4,p)=128 partitions,
    g=32 groups, l=256] so the weighted peer-reduce runs on TensorE.
  - All 16 metric-band stationaries are prefilled by GpSimd at kernel start
    from the static tensor (zeros memset once, diagonal band copied in),
    so z-matmuls never wait on band fills.
  - z via PE with 4x column-group packing: band stationary for group g is
    [128,32] fp8 confined to col-window 32j (j=g//8); 4 groups run
    concurrently via tile_position=(0,32j) into disjoint 32-partition
    slices of psum_z, 8 accumulation waves cover all 32 groups.
  - The per-tile dependency chain (z -> evac -> transpose -> evac -> final
    matmul) is software-pipelined with a 2-iteration skew so the PE
    sequencer never head-of-line blocks on same-tile ACT evacuations:
    iteration t issues my-MM/z-waves(t), transposes(t-1), finals(t-2).
  - my-part via host-transposed myT bf16 stationary vs WmT bf16 moving;
    s1 scaling via ACT per-partition scale on evacuation; z-part via
    PE-transposed zT bf16 vs WpT bf16; s2*wm'+s1*bias' via a K=2 matmul
    with PE-transposed s12; final add on DVE, bf16 out DMA'd every 2
    tiles; host upcasts to f32.
"""

import ml_dtypes
import numpy as np

import concourse.bass as bass
import concourse.mybir as mybir
import concourse.tile as tile
from concourse import bacc
from concourse.bass_utils import run_bass_kernel_spmd

F32 = mybir.dt.float32
BF16 = mybir.dt.bfloat16
FP8 = mybir.dt.float8e3
U8 = mybir.dt.uint8
NP_FP8 = ml_dtypes.float8_e3m4
NP_BF16 = ml_dtypes.bfloat16

B, P, L, R = 16384, 32, 256, 256
N_CORES = 8
BC = B // N_CORES          # 2048 batches per core
TILE_B = 128               # batches per SBUF tile
NT = BC // TILE_B          # 16 tiles
G = TILE_B // 4            # 32 groups of 4 batches

# fused per-tile input layout (bytes per partition lane)
X_OFF, X_BYTES = 0, G * L                  # 8192: x fp8 [g, l]
MYT_OFF, MYT_BYTES = 8192, 2 * TILE_B * 2  # 512: myT bf16 [2, 128]
MB_OFF, MB_BYTES = 8704, P * 2             # 64: natural m bf16
XBYTES = 8768

# static tensors: w1 = my-matmul operands (tiny, first on sync ring so the
# first my-matmul unblocks early); w2 = everything else (scalar ring).
# w1: WmT bf16 [2,256] @0 (1024B); ones bf16 [128] @1024 lane0 (256B);
#     bias' row bf16 [256] @1280 lane0 (512B)
ONES_OFF = 1024
BROW_OFF = 1280
W1BYTES = 1792
WPT_OFF = 0        # WpT bf16 [2, 256] -> 1024B
ID_OFF = 1024      # identity f32 [128] -> 512B
WMP_OFF = 1536     # wm/P bcast bf16 [256] -> 512B
MT_OFF = 2048      # band-order m fp8, nt*32B
W2BYTES = 2048 + NT * G

_cache = {}


def build_bass(nt=NT, num_devices=N_CORES):
    bc = nt * TILE_B
    nc = bacc.Bacc(
        "TRN2", target_bir_lowering=False, debug=False, num_devices=num_devices
    )

    x_d = nc.dram_tensor("xin", [nt, TILE_B, XBYTES], U8, kind="ExternalInput")
    w1_d = nc.dram_tensor("wst1", [TILE_B, W1BYTES], U8, kind="ExternalInput")
    w2_d = nc.dram_tensor("wst2", [TILE_B, W2BYTES], U8, kind="ExternalInput")
    out_d = nc.dram_tensor("out", [bc, R], BF16, kind="ExternalOutput")

    with TileCtx(nc) as (tc, ctx):
        singles = ctx.enter_context(tc.tile_pool(name="singles", bufs=1))
        xp = ctx.enter_context(tc.tile_pool(name="xp", bufs=10))
        small = ctx.enter_context(tc.tile_pool(name="small", bufs=4))
        ztp = ctx.enter_context(tc.tile_pool(name="ztp", bufs=3))
        op = ctx.enter_context(tc.tile_pool(name="op", bufs=3))
        psz = ctx.enter_context(tc.tile_pool(name="psz", bufs=2, space="PSUM"))
        pst = ctx.enter_context(tc.tile_pool(name="pst", bufs=2, space="PSUM"))
        psmy = ctx.enter_context(tc.tile_pool(name="psmy", bufs=2, space="PSUM"))
        pso = ctx.enter_context(tc.tile_pool(name="pso", bufs=2, space="PSUM"))

        # WmT goes FIRST on the sync ring (tiny; the first my-matmul waits on
        # it); the rest of the statics load in parallel on the scalar ring.
        w1_sb = singles.tile([TILE_B, W1BYTES], U8)
        nc.sync.dma_start(out=w1_sb, in_=w1_d[:, :])
        w2_sb = singles.tile([TILE_B, W2BYTES], U8)
        nc.scalar.dma_start(out=w2_sb, in_=w2_d[:, :])
        wmT = w1_sb[:, 0:1024].bitcast(BF16)                     # [128, 512]
        ones1 = w1_sb[0:1, ONES_OFF:ONES_OFF + 256].bitcast(BF16)  # [1, 128]
        brow = w1_sb[0:1, BROW_OFF:BROW_OFF + 512].bitcast(BF16)   # [1, 256]
        wpT = w2_sb[:, WPT_OFF:WPT_OFF + 1024].bitcast(BF16)     # [128, 512]
        ident = w2_sb[:, ID_OFF:ID_OFF + 512].bitcast(F32)       # [128, 128]
        wmP = w2_sb[:, WMP_OFF:WMP_OFF + 512].bitcast(BF16)      # [128, 256]
        mt_all = w2_sb[:, MT_OFF:MT_OFF + NT * G].bitcast(FP8)   # [128, nt*32]

        # Prefill ALL band stationaries up front on GpSimd: band[t][:, g, :]
        # is [128, 32] fp8 whose only nonzeros sit at
        # (b4*32+p, 4*(g%8)+b4) = m[g*4+b4, p]; zeros memset once.
        bands = []
        for i in range(nt):
            band_i = singles.tile([TILE_B, G, 32], FP8, tag=f"band{i}")
            nc.gpsimd.memset(band_i.bitcast(F32), 0.0)
            bands.append(band_i)
        for i in range(nt):
            for b4 in range(4):
                view = bands[i][b4 * P:(b4 + 1) * P, :, :]
                out_ap = bass.AP(
                    tensor=view.tensor, offset=view.offset + b4,
                    ap=[view.ap[0], [256, 4], [36, 8]],
                )
                in_v = mt_all[b4 * P:(b4 + 1) * P, i * G:(i + 1) * G]
                in_ap = bass.AP(
                    tensor=in_v.tensor, offset=in_v.offset,
                    ap=[in_v.ap[0], [8, 4], [1, 8]],
                )
                nc.gpsimd.tensor_copy(out=out_ap, in_=in_ap)

        # Per-tile state carried across the software-pipeline skew.
        st = [dict() for _ in range(nt)]

        def stage_a(t):
            """Tile t: load, my-part matmuls, z-waves, s1/s2, evacs."""
            xt = xp.tile([TILE_B, XBYTES], U8, tag="xt")
            nc.sync.dma_start(out=xt, in_=x_d[t])
            x_v = xt[:, X_OFF:X_OFF + X_BYTES].bitcast(FP8)
            myT_v = xt[:, MYT_OFF:MYT_OFF + MYT_BYTES].bitcast(BF16)
            mb_v = xt[:, MB_OFF:MB_OFF + MB_BYTES].bitcast(BF16)

            # psum_my = my@WmT' + bias' (bias via K=1 ones-row; the s1 scale
            # on evacuation then yields s1*my_part + s1*bias' in one go)
            psum_my = psmy.tile([TILE_B, R], F32, tag="psum_my")
            for c in range(2):
                nc.tensor.matmul(
                    out=psum_my, lhsT=myT_v[:, c * TILE_B:(c + 1) * TILE_B],
                    rhs=wmT[:, c * R:(c + 1) * R],
                    start=(c == 0), stop=False,
                )
            nc.tensor.matmul(
                out=psum_my, lhsT=ones1, rhs=brow, start=False, stop=True,
            )

            band = bands[t]
            psum_z = psz.tile([TILE_B, L], F32, tag="psum_z")
            for w in range(8):
                for j in range(4):
                    g = 8 * j + w
                    nc.tensor.matmul(
                        out=psum_z[32 * j:32 * j + 32, :],
                        lhsT=band[:, g, :],
                        rhs=x_v[:, g * L:(g + 1) * L],
                        start=(w == 0), stop=(w == 7),
                        tile_position=(0, 32 * j),
                    )

            s12 = small.tile([TILE_B, 2], F32, tag="s12")  # [s2 | s1]
            m2 = small.tile([TILE_B, P], F32, tag="m2")
            nc.vector.tensor_mul(m2, mb_v, mb_v)
            nc.vector.tensor_reduce(
                out=s12[:, 0:1], in_=m2, axis=mybir.AxisListType.X,
                op=mybir.AluOpType.add,
            )
            nc.vector.tensor_reduce(
                out=s12[:, 1:2], in_=mb_v, axis=mybir.AxisListType.X,
                op=mybir.AluOpType.add,
            )

            z_sb = small.tile([TILE_B, L], F32, tag="z_sb")
            nc.scalar.copy(out=z_sb, in_=psum_z)
            # my-part scaled by s1 on evacuation (frees psum_my this iter)
            myp = small.tile([TILE_B, R], F32, tag="myp")
            nc.scalar.activation(
                out=myp, in_=psum_my,
                func=mybir.ActivationFunctionType.Copy, scale=s12[:, 1:2],
            )
            st[t].update(z_sb=z_sb, myp=myp, s12=s12)

        def stage_b(t):
            """Tile t: transpose z chunks on PE, evacuate as bf16 stationary."""
            z_sb = st[t]["z_sb"]
            zT = ztp.tile([TILE_B, 2, TILE_B], BF16, tag="zT")
            for c in range(2):
                pt = pst.tile([TILE_B, TILE_B], F32, tag="pt")
                nc.tensor.transpose(
                    out=pt, in_=z_sb[:, c * TILE_B:(c + 1) * TILE_B],
                    identity=ident,
                )
                nc.scalar.copy(out=zT[:, c, :], in_=pt)
            st[t]["zT"] = zT

        def stage_c(t):
            """Tile t: z/tail matmuls, final combine, output DMA."""
            zT = st[t]["zT"]
            psum_o = pso.tile([TILE_B, R], F32, tag="psum_o")
            for c in range(2):
                nc.tensor.matmul(
                    out=psum_o, lhsT=zT[:, c, :],
                    rhs=wpT[:, c * R:(c + 1) * R],
                    start=(c == 0), stop=(c == 1),
                )
            # tail on DVE (2 iterations off the critical path via the skew)
            s12 = st[t]["s12"]
            t2 = small.tile([TILE_B, R], F32, tag="t2")
            nc.vector.scalar_tensor_tensor(
                out=t2, in0=wmP, scalar=s12[:, 0:1], in1=st[t]["myp"],
                op0=mybir.AluOpType.mult, op1=mybir.AluOpType.add,
            )
            if t % 2 == 0:
                st[t]["out2"] = op.tile(
                    [TILE_B, 2, R], BF16, tag="out2", name="out2"
                )
            out2 = st[t - t % 2]["out2"]
            nc.vector.tensor_add(out2[:, t % 2, :], t2, psum_o)
            if t % 2 == 1:
                k = t // 2
                dst = out_d[k * 256:(k + 1) * 256, :].rearrange(
                    "(j u) r -> u j r", j=2
                )
                nc.scalar.dma_start(out=dst, in_=out2)

        for t in range(nt + 2):
            if t < nt:
                stage_a(t)
            if 1 <= t <= nt:
                stage_b(t - 1)
            if t >= 2:
                stage_c(t - 2)

    nc.compile()
    return nc


class TileCtx:
    """with TileCtx(nc) as (tc, ctx): — TileContext plus an ExitStack."""

    def __init__(self, nc):
        from contextlib import ExitStack
        self.tc = tile.TileContext(nc)
        self.ctx = ExitStack()

    def __enter__(self):
        return self.tc.__enter__(), self.ctx.__enter__()

    def __exit__(self, *a):
        self.ctx.__exit__(*a)
        return self.tc.__exit__(*a)


def prep_inputs(my_latent, peer_latents, peer_metrics, W, b):
    """Host-side shard + layout prep (dtype cast / permute / weight packing)."""
    wmT = np.ascontiguousarray(W[:, :L].T / P).astype(NP_BF16)      # [256,256]
    wpT = np.ascontiguousarray(W[:, L:2 * L].T / P).astype(NP_BF16)
    wst1 = np.zeros((TILE_B, W1BYTES), dtype=np.uint8)
    wst1[:, 0:1024] = np.ascontiguousarray(
        wmT.reshape(2, TILE_B, R).transpose(1, 0, 2)
    ).reshape(TILE_B, 2 * R).view(np.uint8)
    wst1[0, ONES_OFF:ONES_OFF + 256] = np.ones(
        TILE_B, dtype=NP_BF16
    ).view(np.uint8)
    wst1[0, BROW_OFF:BROW_OFF + 512] = (b / P).astype(NP_BF16).view(np.uint8)
    wst_common = np.zeros((TILE_B, W2BYTES), dtype=np.uint8)
    wst_common[:, WPT_OFF:WPT_OFF + 1024] = np.ascontiguousarray(
        wpT.reshape(2, TILE_B, R).transpose(1, 0, 2)
    ).reshape(TILE_B, 2 * R).view(np.uint8)
    wst_common[:, ID_OFF:ID_OFF + 512] = np.eye(
        TILE_B, dtype=np.float32
    ).view(np.uint8).reshape(TILE_B, 512)
    wst_common[:, WMP_OFF:WMP_OFF + 512] = np.broadcast_to(
        (W[:, 2 * L] / P).astype(NP_BF16).view(np.uint8), (TILE_B, 512)
    )

    x8_all = np.clip(peer_latents, -15.5, 15.5).astype(NP_FP8)
    myT_all = my_latent.astype(NP_BF16)
    in_maps = []
    for c in range(N_CORES):
        sl = slice(c * BC, (c + 1) * BC)
        xin = np.empty((NT, TILE_B, XBYTES), dtype=np.uint8)
        # x: [(b4,p)=128 partitions, g, l], one contiguous block per tile
        x8 = x8_all[sl].reshape(NT, G, 4, P, L).transpose(0, 2, 3, 1, 4)
        xin[:, :, X_OFF:X_OFF + X_BYTES] = np.ascontiguousarray(x8).reshape(
            NT, TILE_B, G * L
        ).view(np.uint8)
        # myT: lane v holds my[b, 128c+v] for chunks c=0,1
        myT = myT_all[sl].reshape(NT, TILE_B, 2, TILE_B).transpose(0, 3, 2, 1)
        xin[:, :, MYT_OFF:MYT_OFF + MYT_BYTES] = np.ascontiguousarray(
            myT
        ).reshape(NT, TILE_B, 2 * TILE_B).view(np.uint8)
        mc = peer_metrics[sl].astype(np.float32)
        xin[:, :, MB_OFF:MB_OFF + MB_BYTES] = np.ascontiguousarray(
            mc.reshape(NT, TILE_B, P).astype(NP_BF16)
        ).view(np.uint8)
        # band-order metrics, fp8, all tiles -> static tensor:
        # mt[t][b4*32+p, g] = m[4g+b4, p]
        wst2 = wst_common.copy()
        mt = mc.reshape(NT, G, 4, P).transpose(0, 2, 3, 1).astype(NP_FP8)
        wst2[:, MT_OFF:MT_OFF + NT * G] = np.ascontiguousarray(
            mt.reshape(NT, TILE_B, G).transpose(1, 0, 2)
        ).reshape(TILE_B, NT * G).view(np.uint8)
        in_maps.append({"xin": xin, "wst1": wst1, "wst2": wst2})
    return in_maps


def run(my_latent, peer_latents, peer_metrics, W, b, trace=False, **kw):
    if "nc" not in _cache:
        _cache["nc"] = build_bass()
    nc = _cache["nc"]
    in_maps = prep_inputs(
        np.asarray(my_latent, dtype=np.float32),
        np.asarray(peer_latents, dtype=np.float32),
        np.asarray(peer_metrics, dtype=np.float32),
        np.asarray(W, dtype=np.float32),
        np.asarray(b, dtype=np.float32),
    )
    res = run_bass_kernel_spmd(
        nc, in_maps, core_ids=list(range(N_CORES)), trace=trace, **kw
    )
    out = np.concatenate(
        [np.asarray(r["out"]).astype(np.float32) for r in res.results], axis=0
    )
    return out, res


def kernel(my_latent, peer_latents, peer_metrics, W, b):
    out, _ = run(my_latent, peer_latents, peer_metrics, W, b)
    return out


# revision 57
# speedup vs baseline: 1.0077x; 1.0077x over previous
"""Trainium2 Bass kernel for nn_MiddleOut (gnn_message_passing).

Math (reference):
    out[b,r] = mean_p[ m[b,p] * (my@Wm.T + bias + peer[b,p]@Wp.T + m[b,p]*wm)[r] ]
Collapses to (P = #peers):
    s1[b] = sum_p m[b,p];  s2[b] = sum_p m[b,p]^2
    z[b,l] = sum_p m[b,p] * peer[b,p,l]
    out = s1*(my@WmT') + z@WpT' + s2*wm' + s1*bias'     (W' = W/P)

Sharding: pure data parallel over batch across 8 cores.

On-device strategy per core (Bc=2048 rows, 16 tiles of 128):
  - One fused ~1.1MB DMA per tile: [x fp8e3 | myT bf16 | m f32] packed per
    partition lane on host; x host-permuted to [(b4,p)=128 partitions,
    g=32 groups, l=256] so the weighted peer-reduce runs on TensorE.
  - All 16 metric-band stationaries are prefilled by GpSimd at kernel start
    from the static tensor (zeros memset once, diagonal band copied in),
    so z-matmuls never wait on band fills.
  - z via PE with 4x column-group packing: band stationary for group g is
    [128,32] fp8 confined to col-window 32j (j=g//8); 4 groups run
    concurrently via tile_position=(0,32j) into disjoint 32-partition
    slices of psum_z, 8 accumulation waves cover all 32 groups.
  - The per-tile dependency chain (z -> evac -> transpose -> evac -> final
    matmul) is software-pipelined with a 2-iteration skew so the PE
    sequencer never head-of-line blocks on same-tile ACT evacuations:
    iteration t issues my-MM/z-waves(t), transposes(t-1), finals(t-2).
  - my-part via host-transposed myT bf16 stationary vs WmT bf16 moving;
    s1 scaling via ACT per-partition scale on evacuation; z-part via
    PE-transposed zT bf16 vs WpT bf16; s2*wm'+s1*bias' via a K=2 matmul
    with PE-transposed s12; final add on DVE, bf16 out DMA'd every 2
    tiles; host upcasts to f32.
"""

import ml_dtypes
import numpy as np

import concourse.bass as bass
import concourse.mybir as mybir
import concourse.tile as tile
from concourse import bacc
from concourse.bass_utils import run_bass_kernel_spmd

F32 = mybir.dt.float32
BF16 = mybir.dt.bfloat16
FP8 = mybir.dt.float8e3
U8 = mybir.dt.uint8
NP_FP8 = ml_dtypes.float8_e3m4
NP_BF16 = ml_dtypes.bfloat16

B, P, L, R = 16384, 32, 256, 256
N_CORES = 8
BC = B // N_CORES          # 2048 batches per core
TILE_B = 128               # batches per SBUF tile
NT = BC // TILE_B          # 16 tiles
G = TILE_B // 4            # 32 groups of 4 batches

# fused per-tile input layout (bytes per partition lane)
X_OFF, X_BYTES = 0, G * L                  # 8192: x fp8 [g, l]
MYT_OFF, MYT_BYTES = 8192, 2 * TILE_B * 2  # 512: myT bf16 [2, 128]
MB_OFF, MB_BYTES = 8704, P * 2             # 64: natural m bf16
XBYTES = 8768

# static tensors: w1 = my-matmul operands (tiny, first on sync ring so the
# first my-matmul unblocks early); w2 = everything else (scalar ring).
# w1: WmT bf16 [2,256] @0 (1024B); ones bf16 [128] @1024 lane0 (256B);
#     bias' row bf16 [256] @1280 lane0 (512B)
ONES_OFF = 1024
BROW_OFF = 1280
W1BYTES = 1792
WPT_OFF = 0        # WpT bf16 [2, 256] -> 1024B
ID_OFF = 1024      # identity f32 [128] -> 512B
WMP_OFF = 1536     # wm/P bcast bf16 [256] -> 512B
MT_OFF = 2048      # band-order m fp8, nt*32B
W2BYTES = 2048 + NT * G

_cache = {}


def build_bass(nt=NT, num_devices=N_CORES):
    bc = nt * TILE_B
    nc = bacc.Bacc(
        "TRN2", target_bir_lowering=False, debug=False, num_devices=num_devices
    )

    x_d = nc.dram_tensor("xin", [nt, TILE_B, XBYTES], U8, kind="ExternalInput")
    w1_d = nc.dram_tensor("wst1", [TILE_B, W1BYTES], U8, kind="ExternalInput")
    w2_d = nc.dram_tensor("wst2", [TILE_B, W2BYTES], U8, kind="ExternalInput")
    out_d = nc.dram_tensor("out", [bc, R], BF16, kind="ExternalOutput")

    with TileCtx(nc) as (tc, ctx):
        singles = ctx.enter_context(tc.tile_pool(name="singles", bufs=1))
        xp = ctx.enter_context(tc.tile_pool(name="xp", bufs=10))
        small = ctx.enter_context(tc.tile_pool(name="small", bufs=4))
        ztp = ctx.enter_context(tc.tile_pool(name="ztp", bufs=3))
        op = ctx.enter_context(tc.tile_pool(name="op", bufs=3))
        psz = ctx.enter_context(tc.tile_pool(name="psz", bufs=2, space="PSUM"))
        pst = ctx.enter_context(tc.tile_pool(name="pst", bufs=2, space="PSUM"))
        psmy = ctx.enter_context(tc.tile_pool(name="psmy", bufs=2, space="PSUM"))
        pso = ctx.enter_context(tc.tile_pool(name="pso", bufs=2, space="PSUM"))

        # WmT goes FIRST on the sync ring (tiny; the first my-matmul waits on
        # it); the rest of the statics load in parallel on the scalar ring.
        w1_sb = singles.tile([TILE_B, W1BYTES], U8)
        nc.sync.dma_start(out=w1_sb, in_=w1_d[:, :])
        w2_sb = singles.tile([TILE_B, W2BYTES], U8)
        nc.scalar.dma_start(out=w2_sb, in_=w2_d[:, :])
        wmT = w1_sb[:, 0:1024].bitcast(BF16)                     # [128, 512]
        ones1 = w1_sb[0:1, ONES_OFF:ONES_OFF + 256].bitcast(BF16)  # [1, 128]
        brow = w1_sb[0:1, BROW_OFF:BROW_OFF + 512].bitcast(BF16)   # [1, 256]
        wpT = w2_sb[:, WPT_OFF:WPT_OFF + 1024].bitcast(BF16)     # [128, 512]
        ident = w2_sb[:, ID_OFF:ID_OFF + 512].bitcast(F32)       # [128, 128]
        wmP = w2_sb[:, WMP_OFF:WMP_OFF + 512].bitcast(BF16)      # [128, 256]
        mt_all = w2_sb[:, MT_OFF:MT_OFF + NT * G].bitcast(FP8)   # [128, nt*32]

        # Prefill ALL band stationaries up front on GpSimd: band[t][:, g, :]
        # is [128, 32] fp8 whose only nonzeros sit at
        # (b4*32+p, 4*(g%8)+b4) = m[g*4+b4, p]; zeros memset once.
        bands = []
        for i in range(nt):
            band_i = singles.tile([TILE_B, G, 32], FP8, tag=f"band{i}")
            nc.gpsimd.memset(band_i.bitcast(F32), 0.0)
            bands.append(band_i)
        for i in range(nt):
            for b4 in range(4):
                view = bands[i][b4 * P:(b4 + 1) * P, :, :]
                out_ap = bass.AP(
                    tensor=view.tensor, offset=view.offset + b4,
                    ap=[view.ap[0], [256, 4], [36, 8]],
                )
                in_v = mt_all[b4 * P:(b4 + 1) * P, i * G:(i + 1) * G]
                in_ap = bass.AP(
                    tensor=in_v.tensor, offset=in_v.offset,
                    ap=[in_v.ap[0], [8, 4], [1, 8]],
                )
                nc.gpsimd.tensor_copy(out=out_ap, in_=in_ap)

        # Per-tile state carried across the software-pipeline skew.
        st = [dict() for _ in range(nt)]

        def stage_a(t):
            """Tile t: load, my-part matmuls, z-waves, s1/s2, evacs."""
            xt = xp.tile([TILE_B, XBYTES], U8, tag="xt")
            nc.sync.dma_start(out=xt, in_=x_d[t])
            x_v = xt[:, X_OFF:X_OFF + X_BYTES].bitcast(FP8)
            myT_v = xt[:, MYT_OFF:MYT_OFF + MYT_BYTES].bitcast(BF16)
            mb_v = xt[:, MB_OFF:MB_OFF + MB_BYTES].bitcast(BF16)

            # psum_my = my@WmT' + bias' (bias via K=1 ones-row; the s1 scale
            # on evacuation then yields s1*my_part + s1*bias' in one go)
            psum_my = psmy.tile([TILE_B, R], F32, tag="psum_my")
            for c in range(2):
                nc.tensor.matmul(
                    out=psum_my, lhsT=myT_v[:, c * TILE_B:(c + 1) * TILE_B],
                    rhs=wmT[:, c * R:(c + 1) * R],
                    start=(c == 0), stop=False,
                )
            nc.tensor.matmul(
                out=psum_my, lhsT=ones1, rhs=brow, start=False, stop=True,
            )

            band = bands[t]
            psum_z = psz.tile([TILE_B, L], F32, tag="psum_z")
            for w in range(8):
                for j in range(4):
                    g = 8 * j + w
                    nc.tensor.matmul(
                        out=psum_z[32 * j:32 * j + 32, :],
                        lhsT=band[:, g, :],
                        rhs=x_v[:, g * L:(g + 1) * L],
                        start=(w == 0), stop=(w == 7),
                        tile_position=(0, 32 * j),
                    )

            s12 = small.tile([TILE_B, 2], F32, tag="s12")  # [s2 | s1]
            m2 = small.tile([TILE_B, P], F32, tag="m2")
            nc.vector.tensor_mul(m2, mb_v, mb_v)
            nc.vector.tensor_reduce(
                out=s12[:, 0:1], in_=m2, axis=mybir.AxisListType.X,
                op=mybir.AluOpType.add,
            )
            nc.vector.tensor_reduce(
                out=s12[:, 1:2], in_=mb_v, axis=mybir.AxisListType.X,
                op=mybir.AluOpType.add,
            )

            z_sb = small.tile([TILE_B, L], F32, tag="z_sb")
            nc.scalar.copy(out=z_sb, in_=psum_z)
            # my-part scaled by s1 on evacuation (frees psum_my this iter)
            myp = small.tile([TILE_B, R], F32, tag="myp")
            nc.scalar.activation(
                out=myp, in_=psum_my,
                func=mybir.ActivationFunctionType.Copy, scale=s12[:, 1:2],
            )
            st[t].update(z_sb=z_sb, myp=myp, s12=s12)

        def stage_b(t):
            """Tile t: transpose z chunks on PE, evacuate as bf16 stationary."""
            z_sb = st[t]["z_sb"]
            zT = ztp.tile([TILE_B, 2, TILE_B], BF16, tag="zT")
            for c in range(2):
                pt = pst.tile([TILE_B, TILE_B], F32, tag="pt")
                nc.tensor.transpose(
                    out=pt, in_=z_sb[:, c * TILE_B:(c + 1) * TILE_B],
                    identity=ident,
                )
                nc.scalar.copy(out=zT[:, c, :], in_=pt)
            st[t]["zT"] = zT

        def stage_c(t):
            """Tile t: z/tail matmuls, final combine, output DMA."""
            zT = st[t]["zT"]
            psum_o = pso.tile([TILE_B, R], F32, tag="psum_o")
            for c in range(2):
                nc.tensor.matmul(
                    out=psum_o, lhsT=zT[:, c, :],
                    rhs=wpT[:, c * R:(c + 1) * R],
                    start=(c == 0), stop=(c == 1),
                )
            # tail on DVE (2 iterations off the critical path via the skew)
            s12 = st[t]["s12"]
            t2 = small.tile([TILE_B, R], F32, tag="t2")
            nc.vector.scalar_tensor_tensor(
                out=t2, in0=wmP, scalar=s12[:, 0:1], in1=st[t]["myp"],
                op0=mybir.AluOpType.mult, op1=mybir.AluOpType.add,
            )
            if t % 2 == 0:
                st[t]["out2"] = op.tile(
                    [TILE_B, 2, R], BF16, tag="out2", name="out2"
                )
            out2 = st[t - t % 2]["out2"]
            nc.vector.tensor_add(out2[:, t % 2, :], t2, psum_o)
            if t % 2 == 1:
                k = t // 2
                dst = out_d[k * 256:(k + 1) * 256, :].rearrange(
                    "(j u) r -> u j r", j=2
                )
                nc.scalar.dma_start(out=dst, in_=out2)

        for t in range(nt + 2):
            if t < nt:
                stage_a(t)
            if 1 <= t <= nt:
                stage_b(t - 1)
            if t >= 2:
                stage_c(t - 2)

    nc.compile()
    return nc


class TileCtx:
    """with TileCtx(nc) as (tc, ctx): — TileContext plus an ExitStack."""

    def __init__(self, nc):
        from contextlib import ExitStack
        self.tc = tile.TileContext(nc)
        self.ctx = ExitStack()

    def __enter__(self):
        return self.tc.__enter__(), self.ctx.__enter__()

    def __exit__(self, *a):
        self.ctx.__exit__(*a)
        return self.tc.__exit__(*a)


def prep_inputs(my_latent, peer_latents, peer_metrics, W, b):
    """Host-side shard + layout prep (dtype cast / permute / weight packing)."""
    wmT = np.ascontiguousarray(W[:, :L].T / P).astype(NP_BF16)      # [256,256]
    wpT = np.ascontiguousarray(W[:, L:2 * L].T / P).astype(NP_BF16)
    wst1 = np.zeros((TILE_B, W1BYTES), dtype=np.uint8)
    wst1[:, 0:1024] = np.ascontiguousarray(
        wmT.reshape(2, TILE_B, R).transpose(1, 0, 2)
    ).reshape(TILE_B, 2 * R).view(np.uint8)
    wst1[0, ONES_OFF:ONES_OFF + 256] = np.ones(
        TILE_B, dtype=NP_BF16
    ).view(np.uint8)
    wst1[0, BROW_OFF:BROW_OFF + 512] = (b / P).astype(NP_BF16).view(np.uint8)
    wst_common = np.zeros((TILE_B, W2BYTES), dtype=np.uint8)
    wst_common[:, WPT_OFF:WPT_OFF + 1024] = np.ascontiguousarray(
        wpT.reshape(2, TILE_B, R).transpose(1, 0, 2)
    ).reshape(TILE_B, 2 * R).view(np.uint8)
    wst_common[:, ID_OFF:ID_OFF + 512] = np.eye(
        TILE_B, dtype=np.float32
    ).view(np.uint8).reshape(TILE_B, 512)
    wst_common[:, WMP_OFF:WMP_OFF + 512] = np.broadcast_to(
        (W[:, 2 * L] / P).astype(NP_BF16).view(np.uint8), (TILE_B, 512)
    )

    x8_all = np.clip(peer_latents, -15.5, 15.5).astype(NP_FP8)
    myT_all = my_latent.astype(NP_BF16)
    in_maps = []
    for c in range(N_CORES):
        sl = slice(c * BC, (c + 1) * BC)
        xin = np.empty((NT, TILE_B, XBYTES), dtype=np.uint8)
        # x: [(b4,p)=128 partitions, g, l], one contiguous block per tile
        x8 = x8_all[sl].reshape(NT, G, 4, P, L).transpose(0, 2, 3, 1, 4)
        xin[:, :, X_OFF:X_OFF + X_BYTES] = np.ascontiguousarray(x8).reshape(
            NT, TILE_B, G * L
        ).view(np.uint8)
        # myT: lane v holds my[b, 128c+v] for chunks c=0,1
        myT = myT_all[sl].reshape(NT, TILE_B, 2, TILE_B).transpose(0, 3, 2, 1)
        xin[:, :, MYT_OFF:MYT_OFF + MYT_BYTES] = np.ascontiguousarray(
            myT
        ).reshape(NT, TILE_B, 2 * TILE_B).view(np.uint8)
        mc = peer_metrics[sl].astype(np.float32)
        xin[:, :, MB_OFF:MB_OFF + MB_BYTES] = np.ascontiguousarray(
            mc.reshape(NT, TILE_B, P).astype(NP_BF16)
        ).view(np.uint8)
        # band-order metrics, fp8, all tiles -> static tensor:
        # mt[t][b4*32+p, g] = m[4g+b4, p]
        wst2 = wst_common.copy()
        mt = mc.reshape(NT, G, 4, P).transpose(0, 2, 3, 1).astype(NP_FP8)
        wst2[:, MT_OFF:MT_OFF + NT * G] = np.ascontiguousarray(
            mt.reshape(NT, TILE_B, G).transpose(1, 0, 2)
        ).reshape(TILE_B, NT * G).view(np.uint8)
        in_maps.append({"xin": xin, "wst1": wst1, "wst2": wst2})
    return in_maps


def run(my_latent, peer_latents, peer_metrics, W, b, trace=False, **kw):
    if "nc" not in _cache:
        _cache["nc"] = build_bass()
    nc = _cache["nc"]
    in_maps = prep_inputs(
        np.asarray(my_latent, dtype=np.float32),
        np.asarray(peer_latents, dtype=np.float32),
        np.asarray(peer_metrics, dtype=np.float32),
        np.asarray(W, dtype=np.float32),
        np.asarray(b, dtype=np.float32),
    )
    res = run_bass_kernel_spmd(
        nc, in_maps, core_ids=list(range(N_CORES)), trace=trace, **kw
    )
    out = np.concatenate(
        [np.asarray(r["out"]).astype(np.float32) for r in res.results], axis=0
    )
    return out, res


def kernel(my_latent, peer_latents, peer_metrics, W, b):
    out, _ = run(my_latent, peer_latents, peer_metrics, W, b)
    return out


# revision 63
# speedup vs baseline: 1.0462x; 1.0381x over previous
"""Trainium2 Bass kernel for nn_MiddleOut (gnn_message_passing).

Math (reference):
    out[b,r] = mean_p[ m[b,p] * (my@Wm.T + bias + peer[b,p]@Wp.T + m[b,p]*wm)[r] ]
Collapses to (P = #peers):
    s1[b] = sum_p m[b,p];  s2[b] = sum_p m[b,p]^2
    z[b,l] = sum_p m[b,p] * peer[b,p,l]
    out = s1*(my@WmT') + z@WpT' + s2*wm' + s1*bias'     (W' = W/P)

Sharding: pure data parallel over batch across 8 cores.

On-device strategy per core (Bc=2048 rows, 16 tiles of 128):
  - One fused ~1.1MB DMA per tile: [x fp8e3 | myT bf16 | m f32] packed per
    partition lane on host; x host-permuted to [(b4,p)=128 partitions,
    g=32 groups, l=256] so the weighted peer-reduce runs on TensorE.
  - All 16 metric-band stationaries are prefilled by GpSimd at kernel start
    from the static tensor (zeros memset once, diagonal band copied in),
    so z-matmuls never wait on band fills.
  - z via PE with 4x column-group packing: band stationary for group g is
    [128,32] fp8 confined to col-window 32j (j=g//8); 4 groups run
    concurrently via tile_position=(0,32j) into disjoint 32-partition
    slices of psum_z, 8 accumulation waves cover all 32 groups.
  - The per-tile dependency chain (z -> evac -> transpose -> evac -> final
    matmul) is software-pipelined with a 2-iteration skew so the PE
    sequencer never head-of-line blocks on same-tile ACT evacuations:
    iteration t issues my-MM/z-waves(t), transposes(t-1), finals(t-2).
  - my-part via host-transposed myT bf16 stationary vs WmT bf16 moving;
    s1 scaling via ACT per-partition scale on evacuation; z-part via
    PE-transposed zT bf16 vs WpT bf16; s2*wm'+s1*bias' via a K=2 matmul
    with PE-transposed s12; final add on DVE, bf16 out DMA'd every 2
    tiles; host upcasts to f32.
"""

import ml_dtypes
import numpy as np

import concourse.bass as bass
import concourse.mybir as mybir
import concourse.tile as tile
from concourse import bacc
from concourse.bass_utils import run_bass_kernel_spmd

F32 = mybir.dt.float32
BF16 = mybir.dt.bfloat16
FP8 = mybir.dt.float8e3
U8 = mybir.dt.uint8
NP_FP8 = ml_dtypes.float8_e3m4
NP_BF16 = ml_dtypes.bfloat16

B, P, L, R = 16384, 32, 256, 256
N_CORES = 8
BC = B // N_CORES          # 2048 batches per core
TILE_B = 128               # batches per SBUF tile
NT = BC // TILE_B          # 16 tiles
G = TILE_B // 4            # 32 groups of 4 batches

# fused per-tile input layout (bytes per partition lane)
X_OFF, X_BYTES = 0, G * L                  # 8192: x fp8 [g, l]
MYT_OFF, MYT_BYTES = 8192, 2 * TILE_B * 2  # 512: myT bf16 [2, 128]
MB_OFF, MB_BYTES = 8704, P * 2             # 64: natural m bf16
XBYTES = 8768

# static tensors: w1 = just WmT (tiny, first on sync ring so the first
# my-matmul unblocks early); w2 = everything else (parallel, scalar ring)
W1BYTES = 1024     # WmT bf16 [2, 256]
WPT_OFF = 0        # WpT bf16 [2, 256] -> 1024B
ID_OFF = 1024      # identity f32 [128] -> 512B
WMP_OFF = 1536     # wm/P bcast bf16 [256] -> 512B
BIASP_OFF = 2048   # bias/P bcast bf16 [256] -> 512B
MT_OFF = 2560      # band-order m fp8, nt*32B
W2BYTES = 2560 + NT * G

_cache = {}


def build_bass(nt=NT, num_devices=N_CORES):
    bc = nt * TILE_B
    nc = bacc.Bacc(
        "TRN2", target_bir_lowering=False, debug=False, num_devices=num_devices
    )

    x_d = nc.dram_tensor("xin", [nt, TILE_B, XBYTES], U8, kind="ExternalInput")
    w1_d = nc.dram_tensor("wst1", [TILE_B, W1BYTES], U8, kind="ExternalInput")
    w2_d = nc.dram_tensor("wst2", [TILE_B, W2BYTES], U8, kind="ExternalInput")
    out_d = nc.dram_tensor("out", [bc, R], BF16, kind="ExternalOutput")

    with TileCtx(nc) as (tc, ctx):
        singles = ctx.enter_context(tc.tile_pool(name="singles", bufs=1))
        xp = ctx.enter_context(tc.tile_pool(name="xp", bufs=10))
        small = ctx.enter_context(tc.tile_pool(name="small", bufs=4))
        ztp = ctx.enter_context(tc.tile_pool(name="ztp", bufs=3))
        op = ctx.enter_context(tc.tile_pool(name="op", bufs=3))
        psz = ctx.enter_context(tc.tile_pool(name="psz", bufs=2, space="PSUM"))
        pst = ctx.enter_context(tc.tile_pool(name="pst", bufs=2, space="PSUM"))
        psmy = ctx.enter_context(tc.tile_pool(name="psmy", bufs=2, space="PSUM"))
        pso = ctx.enter_context(tc.tile_pool(name="pso", bufs=2, space="PSUM"))

        # WmT goes FIRST on the sync ring (tiny; the first my-matmul waits on
        # it); the rest of the statics load in parallel on the scalar ring.
        w1_sb = singles.tile([TILE_B, W1BYTES], U8)
        nc.sync.dma_start(out=w1_sb, in_=w1_d[:, :])
        w2_sb = singles.tile([TILE_B, W2BYTES], U8)
        nc.scalar.dma_start(out=w2_sb, in_=w2_d[:, :])
        wmT = w1_sb[:, 0:1024].bitcast(BF16)                     # [128, 512]
        wpT = w2_sb[:, WPT_OFF:WPT_OFF + 1024].bitcast(BF16)     # [128, 512]
        ident = w2_sb[:, ID_OFF:ID_OFF + 512].bitcast(F32)       # [128, 128]
        wmP = w2_sb[:, WMP_OFF:WMP_OFF + 512].bitcast(BF16)      # [128, 256]
        biasP = w2_sb[:, BIASP_OFF:BIASP_OFF + 512].bitcast(BF16)
        mt_all = w2_sb[:, MT_OFF:MT_OFF + NT * G].bitcast(FP8)   # [128, nt*32]

        # Prefill ALL band stationaries up front on GpSimd: band[t][:, g, :]
        # is [128, 32] fp8 whose only nonzeros sit at
        # (b4*32+p, 4*(g%8)+b4) = m[g*4+b4, p]; zeros memset once.
        bands = []
        for i in range(nt):
            band_i = singles.tile([TILE_B, G, 32], FP8, tag=f"band{i}")
            nc.gpsimd.memset(band_i.bitcast(F32), 0.0)
            bands.append(band_i)
        for i in range(nt):
            for b4 in range(4):
                view = bands[i][b4 * P:(b4 + 1) * P, :, :]
                out_ap = bass.AP(
                    tensor=view.tensor, offset=view.offset + b4,
                    ap=[view.ap[0], [256, 4], [36, 8]],
                )
                in_v = mt_all[b4 * P:(b4 + 1) * P, i * G:(i + 1) * G]
                in_ap = bass.AP(
                    tensor=in_v.tensor, offset=in_v.offset,
                    ap=[in_v.ap[0], [8, 4], [1, 8]],
                )
                nc.gpsimd.tensor_copy(out=out_ap, in_=in_ap)

        # Per-tile state carried across the software-pipeline skew.
        st = [dict() for _ in range(nt)]

        def stage_a(t):
            """Tile t: load, my-part matmuls, z-waves, s1/s2, evacs."""
            xt = xp.tile([TILE_B, XBYTES], U8, tag="xt")
            nc.sync.dma_start(out=xt, in_=x_d[t])
            x_v = xt[:, X_OFF:X_OFF + X_BYTES].bitcast(FP8)
            myT_v = xt[:, MYT_OFF:MYT_OFF + MYT_BYTES].bitcast(BF16)
            mb_v = xt[:, MB_OFF:MB_OFF + MB_BYTES].bitcast(BF16)

            psum_my = psmy.tile([TILE_B, R], F32, tag="psum_my")
            for c in range(2):
                nc.tensor.matmul(
                    out=psum_my, lhsT=myT_v[:, c * TILE_B:(c + 1) * TILE_B],
                    rhs=wmT[:, c * R:(c + 1) * R],
                    start=(c == 0), stop=(c == 1),
                )

            band = bands[t]
            psum_z = psz.tile([TILE_B, L], F32, tag="psum_z")
            for w in range(8):
                for j in range(4):
                    g = 8 * j + w
                    nc.tensor.matmul(
                        out=psum_z[32 * j:32 * j + 32, :],
                        lhsT=band[:, g, :],
                        rhs=x_v[:, g * L:(g + 1) * L],
                        start=(w == 0), stop=(w == 7),
                        tile_position=(0, 32 * j),
                    )

            s12 = small.tile([TILE_B, 2], F32, tag="s12")  # [s2 | s1]
            m2 = small.tile([TILE_B, P], F32, tag="m2")
            nc.vector.tensor_mul(m2, mb_v, mb_v)
            nc.vector.tensor_reduce(
                out=s12[:, 0:1], in_=m2, axis=mybir.AxisListType.X,
                op=mybir.AluOpType.add,
            )
            nc.vector.tensor_reduce(
                out=s12[:, 1:2], in_=mb_v, axis=mybir.AxisListType.X,
                op=mybir.AluOpType.add,
            )

            z_sb = small.tile([TILE_B, L], F32, tag="z_sb")
            nc.scalar.copy(out=z_sb, in_=psum_z)
            # my-part scaled by s1 on evacuation (frees psum_my this iter)
            myp = small.tile([TILE_B, R], F32, tag="myp")
            nc.scalar.activation(
                out=myp, in_=psum_my,
                func=mybir.ActivationFunctionType.Copy, scale=s12[:, 1:2],
            )
            st[t].update(z_sb=z_sb, myp=myp, s12=s12)

        def stage_b(t):
            """Tile t: transpose z chunks on PE, evacuate as bf16 stationary."""
            z_sb = st[t]["z_sb"]
            zT = ztp.tile([TILE_B, 2, TILE_B], BF16, tag="zT")
            for c in range(2):
                pt = pst.tile([TILE_B, TILE_B], F32, tag="pt")
                nc.tensor.transpose(
                    out=pt, in_=z_sb[:, c * TILE_B:(c + 1) * TILE_B],
                    identity=ident,
                )
                nc.scalar.copy(out=zT[:, c, :], in_=pt)
            st[t]["zT"] = zT

        def stage_c(t):
            """Tile t: z/tail matmuls, final combine, output DMA."""
            zT = st[t]["zT"]
            psum_o = pso.tile([TILE_B, R], F32, tag="psum_o")
            for c in range(2):
                nc.tensor.matmul(
                    out=psum_o, lhsT=zT[:, c, :],
                    rhs=wpT[:, c * R:(c + 1) * R],
                    start=(c == 0), stop=(c == 1),
                )
            # tail on DVE (2 iterations off the critical path via the skew)
            s12 = st[t]["s12"]
            t1 = small.tile([TILE_B, R], F32, tag="t1")
            nc.vector.scalar_tensor_tensor(
                out=t1, in0=wmP, scalar=s12[:, 0:1], in1=st[t]["myp"],
                op0=mybir.AluOpType.mult, op1=mybir.AluOpType.add,
            )
            t2 = small.tile([TILE_B, R], F32, tag="t2")
            nc.vector.scalar_tensor_tensor(
                out=t2, in0=biasP, scalar=s12[:, 1:2], in1=t1,
                op0=mybir.AluOpType.mult, op1=mybir.AluOpType.add,
            )
            if t % 2 == 0:
                st[t]["out2"] = op.tile(
                    [TILE_B, 2, R], BF16, tag="out2", name="out2"
                )
            out2 = st[t - t % 2]["out2"]
            nc.vector.tensor_add(out2[:, t % 2, :], t2, psum_o)
            if t % 2 == 1:
                k = t // 2
                dst = out_d[k * 256:(k + 1) * 256, :].rearrange(
                    "(j u) r -> u j r", j=2
                )
                nc.scalar.dma_start(out=dst, in_=out2)

        for t in range(nt + 2):
            if t < nt:
                stage_a(t)
            if 1 <= t <= nt:
                stage_b(t - 1)
            if t >= 2:
                stage_c(t - 2)

    nc.compile()
    return nc


class TileCtx:
    """with TileCtx(nc) as (tc, ctx): — TileContext plus an ExitStack."""

    def __init__(self, nc):
        from contextlib import ExitStack
        self.tc = tile.TileContext(nc)
        self.ctx = ExitStack()

    def __enter__(self):
        return self.tc.__enter__(), self.ctx.__enter__()

    def __exit__(self, *a):
        self.ctx.__exit__(*a)
        return self.tc.__exit__(*a)


def prep_inputs(my_latent, peer_latents, peer_metrics, W, b):
    """Host-side shard + layout prep (dtype cast / permute / weight packing)."""
    wmT = np.ascontiguousarray(W[:, :L].T / P).astype(NP_BF16)      # [256,256]
    wpT = np.ascontiguousarray(W[:, L:2 * L].T / P).astype(NP_BF16)
    wst1 = np.ascontiguousarray(
        wmT.reshape(2, TILE_B, R).transpose(1, 0, 2)
    ).reshape(TILE_B, 2 * R).view(np.uint8)
    wst_common = np.zeros((TILE_B, W2BYTES), dtype=np.uint8)
    wst_common[:, WPT_OFF:WPT_OFF + 1024] = np.ascontiguousarray(
        wpT.reshape(2, TILE_B, R).transpose(1, 0, 2)
    ).reshape(TILE_B, 2 * R).view(np.uint8)
    wst_common[:, ID_OFF:ID_OFF + 512] = np.eye(
        TILE_B, dtype=np.float32
    ).view(np.uint8).reshape(TILE_B, 512)
    wst_common[:, WMP_OFF:WMP_OFF + 512] = np.broadcast_to(
        (W[:, 2 * L] / P).astype(NP_BF16).view(np.uint8), (TILE_B, 512)
    )
    wst_common[:, BIASP_OFF:BIASP_OFF + 512] = np.broadcast_to(
        (b / P).astype(NP_BF16).view(np.uint8), (TILE_B, 512)
    )

    x8_all = np.clip(peer_latents, -15.5, 15.5).astype(NP_FP8)
    myT_all = my_latent.astype(NP_BF16)
    in_maps = []
    for c in range(N_CORES):
        sl = slice(c * BC, (c + 1) * BC)
        xin = np.empty((NT, TILE_B, XBYTES), dtype=np.uint8)
        # x: [(b4,p)=128 partitions, g, l], one contiguous block per tile
        x8 = x8_all[sl].reshape(NT, G, 4, P, L).transpose(0, 2, 3, 1, 4)
        xin[:, :, X_OFF:X_OFF + X_BYTES] = np.ascontiguousarray(x8).reshape(
            NT, TILE_B, G * L
        ).view(np.uint8)
        # myT: lane v holds my[b, 128c+v] for chunks c=0,1
        myT = myT_all[sl].reshape(NT, TILE_B, 2, TILE_B).transpose(0, 3, 2, 1)
        xin[:, :, MYT_OFF:MYT_OFF + MYT_BYTES] = np.ascontiguousarray(
            myT
        ).reshape(NT, TILE_B, 2 * TILE_B).view(np.uint8)
        mc = peer_metrics[sl].astype(np.float32)
        xin[:, :, MB_OFF:MB_OFF + MB_BYTES] = np.ascontiguousarray(
            mc.reshape(NT, TILE_B, P).astype(NP_BF16)
        ).view(np.uint8)
        # band-order metrics, fp8, all tiles -> static tensor:
        # mt[t][b4*32+p, g] = m[4g+b4, p]
        wst2 = wst_common.copy()
        mt = mc.reshape(NT, G, 4, P).transpose(0, 2, 3, 1).astype(NP_FP8)
        wst2[:, MT_OFF:MT_OFF + NT * G] = np.ascontiguousarray(
            mt.reshape(NT, TILE_B, G).transpose(1, 0, 2)
        ).reshape(TILE_B, NT * G).view(np.uint8)
        in_maps.append({"xin": xin, "wst1": wst1, "wst2": wst2})
    return in_maps


def run(my_latent, peer_latents, peer_metrics, W, b, trace=False, **kw):
    if "nc" not in _cache:
        _cache["nc"] = build_bass()
    nc = _cache["nc"]
    in_maps = prep_inputs(
        np.asarray(my_latent, dtype=np.float32),
        np.asarray(peer_latents, dtype=np.float32),
        np.asarray(peer_metrics, dtype=np.float32),
        np.asarray(W, dtype=np.float32),
        np.asarray(b, dtype=np.float32),
    )
    res = run_bass_kernel_spmd(
        nc, in_maps, core_ids=list(range(N_CORES)), trace=trace, **kw
    )
    out = np.concatenate(
        [np.asarray(r["out"]).astype(np.float32) for r in res.results], axis=0
    )
    return out, res


def kernel(my_latent, peer_latents, peer_metrics, W, b):
    out, _ = run(my_latent, peer_latents, peer_metrics, W, b)
    return out
